# revision 31
# baseline (speedup 1.0000x reference)
"""Trainium2 Bass kernel for nn_Attention_63745904608049.

Relative-position attention (TransformerXL-style shift, Enformer-style pos
basis), batch 1, seq 2048, d_model 1536, 8 heads x 64. Head-parallel over 8
NeuronCores; the positional-score relative shift is realized as a DRAM
roundtrip (straight write, sheared flat-AP read).

Self-contained: hardcodes shapes, computes the (input-independent) positional
embedding table on host, builds one SPMD Bass graph, runs it on cores 0-7 via
run_bass_kernel_spmd, and reassembles the full output.
"""
import contextlib
import ctypes
import math
import os
import sys
import types

import numpy as np
import ml_dtypes

import concourse.bass as bass
import concourse.mybir as mybir
from concourse.tile import TileContext
from concourse.masks import make_identity
from concourse.bass_utils import run_bass_kernel_spmd

# ----------------------------------------------------------------------------
# problem constants
N = 2048
DM = 1536
H = 8
HD = 64
INNER = H * HD            # 512
NCORES = 8
QT = N // 128             # 16 query tiles
WIN = 2175                # per-q-tile pos table window (128 + 2048 - 1)
TSTRIDE = 2304            # padded row stride of the T scratch (elements)
CHUNKS = [(0, 512), (512, 512), (1024, 512), (1536, 512), (2048, 127)]
F32 = mybir.dt.float32
BF16 = mybir.dt.bfloat16
FP16 = mybir.dt.float16

_LAST_RESULT = None       # BassKernelResults of the last run (for test.py)


# ----------------------------------------------------------------------------
# axon NTFF profiling hook (lets BASS_TRACE=1 produce exec_time_ns under axon)
def _install_ntff_hook(so_path="/opt/axon/libaxon_pjrt.so"):
    try:
        import antenv.axon_hooks  # noqa: F401
        return
    except ImportError:
        pass
    try:
        lib = ctypes.CDLL(so_path)
    except OSError:
        return
    if not hasattr(lib, "axon_start_nrt_profile"):
        return
    lib.axon_start_nrt_profile.argtypes = [ctypes.POINTER(ctypes.c_int64), ctypes.c_size_t]
    lib.axon_start_nrt_profile.restype = ctypes.c_int64
    lib.axon_stop_nrt_profile.argtypes = [ctypes.c_char_p]
    lib.axon_stop_nrt_profile.restype = ctypes.c_int64

    @contextlib.contextmanager
    def _hook(output_dir, device_ids):
        import jax
        jax.devices()
        if device_ids:
            ids = (ctypes.c_int64 * len(device_ids))(*device_ids)
            rc = lib.axon_start_nrt_profile(ids, len(device_ids))
        else:
            rc = lib.axon_start_nrt_profile(None, 0)
        if rc != 0:
            raise RuntimeError(f"axon_start_nrt_profile rc={rc}")
        try:
            yield
        finally:
            n = lib.axon_stop_nrt_profile(str(output_dir).encode())
            print(f"ntff profile: {n} file(s) written to {output_dir}")

    mod = types.ModuleType("antenv.axon_hooks")
    mod.get_axon_ntff_profile_hook = lambda: _hook
    mod.set_axon_ntff_profile_hook = lambda h: None
    sys.modules["antenv.axon_hooks"] = mod


_install_ntff_hook()


# ----------------------------------------------------------------------------
# BIR post-processing: this container's walrus build rejects instructions with
# more than one sync wait; split extra waits onto preceding NoOps.
def _split_waits(bir_bytes, maxw=1):
    import json
    d = json.loads(bir_bytes)
    counter = [0]
    for fn in d["functions"]:
        for blk in fn["blocks"]:
            out = []
            for ins in blk["instructions"]:
                si = ins.get("sync_info")
                waits = (si or {}).get("on_wait") or []
                if len(waits) > maxw:
                    excess = waits[:-maxw]
                    ins["sync_info"]["on_wait"] = waits[-maxw:]
                    for i in range(0, len(excess), maxw):
                        counter[0] += 1
                        nop = {
                            "engine": ins["engine"],
                            "ins": [],
                            "outs": [],
                            "name": f"I-waitsplit-{counter[0]}",
                            "opcode": "NoOp",
                            "sync_info": {"on_update": [],
                                          "on_wait": excess[i:i + maxw]},
                        }
                        if "debug" in ins:
                            nop["debug"] = ins["debug"]
                        out.append(nop)
                out.append(ins)
            blk["instructions"] = out
    return json.dumps(d).encode()


# ----------------------------------------------------------------------------
# host-side positional embedding table (pure function of N, DM)
_POS_CACHE = {}


def _pos_embed():
    if "emb" in _POS_CACHE:
        return _POS_CACHE["emb"]
    n, fs = N, DM
    nb = fs // 6  # 256
    dist = np.arange(-n + 1, n, dtype=np.float64)
    adist = np.abs(dist)[:, None]

    max_range = math.log(n) / math.log(2.0)
    half_life = 2.0 ** np.linspace(3.0, max_range, nb)
    exp_feat = np.exp(-math.log(2.0) / half_life[None, :] * adist)

    with np.errstate(over="ignore"):
        center_widths = 2.0 ** np.arange(1, nb + 1, dtype=np.float64) - 1.0
    cmask_feat = (center_widths[None, :] > adist).astype(np.float64)

    stddev = n / (2.0 * nb)
    start_mean = n / nb
    mean = np.linspace(start_mean, float(n), nb)[None, :]
    conc = (mean / stddev) ** 2
    rate = mean / stddev ** 2
    with np.errstate(divide="ignore", invalid="ignore"):
        log_unnorm = (conc - 1.0) * np.log(adist) - rate * adist
    lgamma = np.vectorize(math.lgamma)
    log_norm = lgamma(conc) - conc * np.log(rate)
    with np.errstate(invalid="ignore"):
        prob = np.exp(log_unnorm - log_norm) + 1e-08
    prob = np.nan_to_num(prob, nan=1e-08)  # adist=0: 0*inf -> use limit 0, then +eps
    # recompute the adist == 0 row exactly: log_unnorm = -inf -> exp -> 0
    zrow = np.where(adist[:, 0] == 0)[0]
    prob[zrow, :] = 1e-08
    gamma_feat = prob / prob.max(axis=-1, keepdims=True)

    emb = np.concatenate([exp_feat, cmask_feat, gamma_feat], axis=-1)
    emb = np.concatenate([emb, np.sign(dist)[:, None] * emb], axis=-1)
    emb = emb.astype(np.float32)  # (4095, 1536)
    _POS_CACHE["emb"] = emb
    return emb


# ----------------------------------------------------------------------------
# device graph (identical for all cores; per-core data differs)
_GRAPH_CACHE = {}


def _build_graph():
    if "nc" in _GRAPH_CACHE:
        return _GRAPH_CACHE["nc"]
    nc = bass.Bass()

    xT = nc.declare_dram_parameter("xT", [DM, N], FP16, isOutput=False)
    wq = nc.declare_dram_parameter("wq", [DM, HD], FP16, isOutput=False)
    wkv = nc.declare_dram_parameter("wkv", [DM, 2 * HD], FP16, isOutput=False)
    wps = nc.declare_dram_parameter("wps", [DM, HD], FP16, isOutput=False)
    posT = nc.declare_dram_parameter("posT", [DM, 512], FP16, isOutput=False)
    cbias = nc.declare_dram_parameter("cbias", [HD, 1], F32, isOutput=False)
    pbias = nc.declare_dram_parameter("pbias", [HD, 1], F32, isOutput=False)
    wpb = nc.declare_dram_parameter("wpb", [HD, 1], F32, isOutput=False)
    wout = nc.declare_dram_parameter("wout", [INNER, 192], BF16, isOutput=False)
    bout = nc.declare_dram_parameter("bout", [1, 192], F32, isOutput=False)
    out_ext = nc.declare_dram_parameter("out", [N, 192], F32, isOutput=True)

    # internal DRAM
    t_dram = [nc.dram_tensor(f"tscratch{a}", [128, TSTRIDE], FP16) for a in range(QT)]
    pag_in = nc.dram_tensor("pag_in", [HD, 512], FP16)
    pag_out = nc.dram_tensor("pag_out", [NCORES * HD, 512], FP16, addr_space="Shared")
    oag_in = [nc.dram_tensor(f"oag_in{h}", [HD, N // 4], BF16) for h in range(4)]
    oag_out = [nc.dram_tensor(f"oag_out{h}", [NCORES * HD, N // 4], BF16,
                              addr_space="Shared") for h in range(4)]

    groups = [list(range(NCORES))]
    Act = mybir.ActivationFunctionType

    with TileContext(nc) as tc:
        with contextlib.ExitStack() as ctx:
            persist = ctx.enter_context(tc.tile_pool(name="persist", bufs=1))
            work = ctx.enter_context(tc.tile_pool(name="work", bufs=2))
            psum = ctx.enter_context(tc.tile_pool(name="psum", bufs=3, space="PSUM"))

            # ---------------- phase 1: projections ----------------
            qcT = persist.tile([HD, N], FP16, tag="qcT")
            qpT = persist.tile([HD, N], FP16, tag="qpT")
            kvT = persist.tile([128, N], FP16, tag="kvT")   # k rows 0:64, v rows 64:128
            PT = persist.tile([HD, 8 * 512], FP16, tag="PT")
            vsb = [persist.tile([128, HD], BF16, tag=f"v{k}", name=f"v{k}") for k in range(QT)]
            ident = persist.tile([128, 128], FP16, tag="ident")
            make_identity(nc, ident)
            identbf = persist.tile([128, 128], BF16, tag="identbf")
            make_identity(nc, identbf)

            cb_sb = persist.tile([HD, 1], F32, tag="cb")
            pb_sb = persist.tile([HD, 1], F32, tag="pb")
            wpb_sb = persist.tile([HD, 1], F32, tag="wpb")
            nc.sync.dma_start(out=cb_sb, in_=cbias[:, :])
            nc.sync.dma_start(out=pb_sb, in_=pbias[:, :])
            nc.sync.dma_start(out=wpb_sb, in_=wpb[:, :])

            with contextlib.ExitStack() as ph1:
                wpool = ph1.enter_context(tc.tile_pool(name="wpool", bufs=1))
                xstream = ph1.enter_context(tc.tile_pool(name="xstream", bufs=3))
                # P^T slice first so its AllGather clears as early as possible
                wps_sb = []
                for f in range(12):
                    t = wpool.tile([128, HD], FP16, tag=f"wps{f}", name=f"wps{f}")
                    nc.sync.dma_start(out=t, in_=wps[128 * f:128 * (f + 1), :])
                    wps_sb.append(t)
                ps = psum.tile([HD, 512], F32, tag="big", bufs=4)
                for f in range(12):
                    post = xstream.tile([128, 512], FP16, tag="pos")
                    nc.sync.dma_start(out=post, in_=posT[128 * f:128 * (f + 1), :])
                    nc.tensor.matmul(ps, wps_sb[f], post, start=(f == 0), stop=(f == 11))
                pag_sb = work.tile([HD, 512], FP16, tag="pag")
                nc.vector.tensor_scalar_add(pag_sb, ps, wpb_sb)
                nc.gpsimd.dma_start(out=pag_in[:, :], in_=pag_sb)
                nc.gpsimd.collective_compute(
                    "AllGather", mybir.AluOpType.bypass, replica_groups=groups,
                    ins=[pag_in.ap().opt()], outs=[pag_out.ap().opt()])
                for r in range(NCORES):
                    nc.gpsimd.dma_start(out=PT[:, 512 * r:512 * (r + 1)],
                                        in_=pag_out[HD * r:HD * (r + 1), :])

                wq_sb, wkv_sb = [], []
                for f in range(12):
                    t = wpool.tile([128, HD], FP16, tag=f"wq{f}", name=f"wq{f}")
                    nc.sync.dma_start(out=t, in_=wq[128 * f:128 * (f + 1), :])
                    wq_sb.append(t)
                    t = wpool.tile([128, 2 * HD], FP16, tag=f"wkv{f}", name=f"wkv{f}")
                    nc.sync.dma_start(out=t, in_=wkv[128 * f:128 * (f + 1), :])
                    wkv_sb.append(t)

                # single-pass projections: 8 accumulating psum banks
                q_ps = [psum.tile([HD, 512], F32, tag="small", bufs=4,
                                  name=f"qps{i}") for i in range(4)]
                kv_ps = [psum.tile([128, 512], F32, tag="big", bufs=4,
                                   name=f"kvps{i}") for i in range(4)]
                for f in range(12):
                    xt = xstream.tile([128, N], FP16, tag="xt")
                    nc.scalar.dma_start(out=xt, in_=xT[128 * f:128 * (f + 1), :])
                    for i in range(4):
                        nc.tensor.matmul(q_ps[i], wq_sb[f],
                                         xt[:, 512 * i:512 * (i + 1)],
                                         start=(f == 0), stop=(f == 11))
                    for i in range(4):
                        nc.tensor.matmul(kv_ps[i], wkv_sb[f],
                                         xt[:, 512 * i:512 * (i + 1)],
                                         start=(f == 0), stop=(f == 11))
                for i in range(4):
                    nc.scalar.activation(qcT[:, 512 * i:512 * (i + 1)], q_ps[i],
                                         Act.Identity, bias=cb_sb)
                    nc.scalar.activation(qpT[:, 512 * i:512 * (i + 1)], q_ps[i],
                                         Act.Identity, bias=pb_sb)
                    nc.scalar.activation(kvT[:, 512 * i:512 * (i + 1)], kv_ps[i],
                                         Act.Copy)
                # v transpose to seq-major bf16
                for k in range(QT):
                    tp = psum.tile([128, HD], FP16, tag="small", bufs=4)
                    nc.tensor.transpose(tp, kvT[HD:128, 128 * k:128 * (k + 1)],
                                        ident[HD:128, HD:128])
                    nc.vector.tensor_copy(vsb[k], tp)

            # ---------------- phase 2a: scores + softmax -> an[a] ----------
            ogT = [persist.tile([HD, N // 4], BF16, tag=f"ogT{h}", name=f"ogT{h}")
                   for h in range(4)]
            an = [persist.tile([128, N], BF16, tag=f"an{a}", name=f"an{a}")
                  for a in range(QT)]
            def produce_t(a):
                """T matmuls -> fp16 cast -> DRAM write -> sheared read."""
                w0 = 1920 - 128 * a
                tsb = work.tile([128, TSTRIDE], FP16, tag="tsb", bufs=3,
                                name=f"tsb{a}")
                for ci, (off, w) in enumerate(CHUNKS):
                    tp = psum.tile([128, 512], F32, tag="big", bufs=4,
                                   name=f"tp{a}_{off}")
                    nc.tensor.matmul(tp[:, :w], qpT[:, 128 * a:128 * (a + 1)],
                                     PT[:, w0 + off:w0 + off + w],
                                     start=True, stop=True)
                    if ci in (0, 4):
                        nc.vector.tensor_copy(tsb[:, off:off + w], tp[:, :w])
                    else:
                        nc.scalar.copy(tsb[:, off:off + w], tp[:, :w])
                nc.gpsimd.dma_start(out=t_dram[a][:, 0:WIN], in_=tsb[:, 0:WIN])
                shear = work.tile([128, N], FP16, tag="shear", bufs=3,
                                  name=f"shear{a}")
                src = bass.AP(tensor=t_dram[a].ap().tensor, offset=127,
                              ap=[[TSTRIDE - 1, 128], [1, N]])
                nc.sync.dma_start(out=shear, in_=src)
                return shear

            def consume_t(a, shear):
                """scores = shear + qk (PSUM accumulate), exp, normalize."""
                a_sb = work.tile([128, N], BF16, tag="a", bufs=3, name=f"asb{a}")
                rs4 = work.tile([128, 4], F32, tag="rs4", name=f"rs4{a}")
                for j in range(4):
                    sp = psum.tile([128, 512], F32, tag="big", bufs=4,
                                   name=f"sp{a}_{j}")
                    nc.tensor.matmul(sp, qcT[:, 128 * a:128 * (a + 1)],
                                     kvT[0:HD, 512 * j:512 * (j + 1)],
                                     start=True, stop=True)
                    nc.vector.tensor_add(sp, sp, shear[:, 512 * j:512 * (j + 1)])
                    nc.scalar.activation(a_sb[:, 512 * j:512 * (j + 1)], sp,
                                         Act.Exp, accum_out=rs4[:, j:j + 1])
                rs = work.tile([128, 1], F32, tag="rs", name=f"rs{a}")
                nc.vector.reduce_sum(rs, rs4, axis=mybir.AxisListType.X)
                rsi = work.tile([128, 1], F32, tag="rsi", name=f"rsi{a}")
                nc.vector.reciprocal(rsi, rs)
                nc.vector.tensor_scalar_mul(an[a], a_sb, rsi)

            wout_sb = []
            for r in range(4):
                t = persist.tile([128, 192], BF16, tag=f"wo{r}", name=f"wo{r}")
                nc.sync.dma_start(out=t, in_=wout[128 * r:128 * (r + 1), :])
                wout_sb.append(t)
            bout_sb = persist.tile([128, 192], F32, tag="bo")
            bout_bc = bass.AP(tensor=bout.ap().tensor, offset=0,
                              ap=[[0, 128], [1, 192]])
            nc.sync.dma_start(out=bout_sb, in_=bout_bc)

            def scores_range(a0, a1):
                LOOKAHEAD = 2
                shears = {}
                for a in range(a0, min(a0 + LOOKAHEAD, a1)):
                    shears[a] = produce_t(a)
                for a in range(a0, a1):
                    if a + LOOKAHEAD < a1:
                        shears[a + LOOKAHEAD] = produce_t(a + LOOKAHEAD)
                    consume_t(a, shears.pop(a))

            def av_range(a0, a1):
                for a in range(a0, a1):
                    at3 = work.tile([128, QT, 128], BF16, tag="at3", bufs=3,
                                    name=f"at3_{a}")
                    nc.sync.dma_start_transpose(at3, an[a])
                    otp = psum.tile([HD, 128], F32, tag="small", bufs=4,
                                    name=f"otp{a}")
                    for k in range(QT):
                        nc.tensor.matmul(otp, vsb[k], at3[:, k, :],
                                         start=(k == 0), stop=(k == QT - 1))
                    nc.scalar.activation(
                        ogT[a // 4][:, 128 * (a % 4):128 * (a % 4 + 1)], otp,
                        Act.Copy)

            def issue_ag(h):
                nc.gpsimd.dma_start(out=oag_in[h][:, :], in_=ogT[h])
                nc.gpsimd.collective_compute(
                    "AllGather", mybir.AluOpType.bypass, replica_groups=groups,
                    ins=[oag_in[h].ap().opt()], outs=[oag_out[h].ap().opt()])

            def fin_chunk(h):
                ofull = []
                for r in range(4):
                    t = persist.tile([128, N // 4], BF16, tag=f"of{h}{r}",
                                     name=f"of{h}{r}")
                    nc.gpsimd.dma_start(out=t,
                                        in_=oag_out[h][128 * r:128 * (r + 1), :])
                    ofull.append(t)
                for mm in range(QT // 4):
                    fp = psum.tile([128, 192], F32, tag="small", bufs=4,
                                   name=f"fp{h}_{mm}")
                    for r in range(4):
                        nc.tensor.matmul(fp, ofull[r][:, 128 * mm:128 * (mm + 1)],
                                         wout_sb[r], start=(r == 0), stop=(r == 3))
                    ob = work.tile([128, 192], F32, tag="ob", name=f"ob{h}_{mm}")
                    nc.vector.tensor_add(ob, fp, bout_sb)
                    m = h * 4 + mm
                    nc.gpsimd.dma_start(out=out_ext[128 * m:128 * (m + 1), :],
                                        in_=ob)

            scores_range(0, QT // 2)
            tc.no_sync_barrier()
            av_range(0, QT // 2)
            tc.no_sync_barrier()
            issue_ag(0)
            issue_ag(1)
            scores_range(QT // 2, QT)
            fin_chunk(0)
            fin_chunk(1)
            tc.no_sync_barrier()
            av_range(QT // 2, QT)
            tc.no_sync_barrier()
            issue_ag(2)
            issue_ag(3)
            fin_chunk(2)
            fin_chunk(3)

    # wait-split post-processing hook
    orig = nc.to_json_bytes
    nc.to_json_bytes = lambda: _split_waits(orig())
    _GRAPH_CACHE["nc"] = nc
    return nc


# ----------------------------------------------------------------------------
def _prep_inputs(x, Wq, Wk, Wv, content_bias, pos_bias, Wp_w, Wp_b, Wout_w, Wout_b):
    x = np.ascontiguousarray(np.asarray(x, dtype=np.float32))
    Wq = np.asarray(Wq, np.float32); Wk = np.asarray(Wk, np.float32)
    Wv = np.asarray(Wv, np.float32)
    content_bias = np.asarray(content_bias, np.float32)
    pos_bias = np.asarray(pos_bias, np.float32)
    Wp_w = np.asarray(Wp_w, np.float32); Wp_b = np.asarray(Wp_b, np.float32)
    Wout_w = np.asarray(Wout_w, np.float32); Wout_b = np.asarray(Wout_b, np.float32)

    scale = HD ** -0.5
    xT = np.ascontiguousarray(x[0].T)                    # (1536, 2048)
    emb = _pos_embed()                                   # (4095, 1536)
    embT_pad = np.zeros((DM, NCORES * 512), np.float32)
    embT_pad[:, :2 * N - 1] = emb.T
    wp_sum = Wp_w.reshape(DM, H, HD).sum(axis=1)         # (1536, 64)
    wp_b_sum = Wp_b.reshape(H, HD).sum(axis=0)           # (64,)
    xT16 = np.ascontiguousarray(xT).astype(np.float16)
    wps16 = np.ascontiguousarray(wp_sum).astype(np.float16)

    in_maps = []
    for c in range(NCORES):
        sl = slice(HD * c, HD * (c + 1))
        in_maps.append({
            "xT": xT16,
            "wq": np.ascontiguousarray(Wq[:, sl] * scale).astype(np.float16),
            "wkv": np.ascontiguousarray(
                np.concatenate([Wk[:, sl], Wv[:, sl]], axis=1)).astype(np.float16),
            "wps": wps16,
            "posT": np.ascontiguousarray(
                embT_pad[:, 512 * c:512 * (c + 1)]).astype(np.float16),
            "cbias": np.ascontiguousarray(content_bias[c, 0, :, None]),
            "pbias": np.ascontiguousarray(pos_bias[c, 0, :, None]),
            "wpb": np.ascontiguousarray(wp_b_sum[:, None]),
            "wout": np.ascontiguousarray(
                Wout_w[:, 192 * c:192 * (c + 1)]).astype(ml_dtypes.bfloat16),
            "bout": np.ascontiguousarray(Wout_b[None, 192 * c:192 * (c + 1)]),
        })
    return in_maps


def kernel(x, Wq, Wk, Wv, content_bias, pos_bias, Wp_w, Wp_b, Wout_w, Wout_b):
    global _LAST_RESULT
    in_maps = _prep_inputs(x, Wq, Wk, Wv, content_bias, pos_bias,
                           Wp_w, Wp_b, Wout_w, Wout_b)
    nc = _build_graph()
    trace = bool(os.environ.get("KERNEL_TRACE"))
    res = run_bass_kernel_spmd(nc, in_maps, core_ids=list(range(NCORES)),
                               trace=trace, trace_cores=[0] if trace else None)
    _LAST_RESULT = res
    out = np.concatenate([res.results[c]["out"] for c in range(NCORES)], axis=1)
    return out[None].astype(np.float32)
